# revision 1
# baseline (speedup 1.0000x reference)
"""Local (7x7 windowed) attention Trainium2 kernel.

Problem: B=1, N=4096 (T=4, H=W=32), C=384, 8 heads x hd=48, window 7x7
zero-padded (reference semantics: padded keys score exactly 0 -> weight
exp(0), value 0).

Sharding: data-parallel over positions. 8 cores; core c owns t-slice
c//2, query rows [16*(c%2), 16*(c%2)+16) (512 queries). Each core
recomputes k/v for a 3-row halo (24 rows = 768 halo positions,
zero-padded outside the image, matching the reference's zero padding).

Device pipeline per core (all matmul operands float32r, 1 cyc/row for
N>=256):
  1. qk^T = w_qk^T @ x^T  -> (128=[q_h 64|k_h 64], 768) per head
     v    = x @ w_v       -> (768, 384), stored as [V|1] blocks (49/head)
  2. per (head, j-tile): S^T = mask_add + K^T.T @ Q^T in PSUM (banded
     i-spans), exp on ACT -> E^T
  3. out' = [V|1].T @ E^T accumulated over j-tiles -> (49+pad, 512) pairs
     row 48 = softmax denominator (+ n_oob via extra matmul)
  4. reciprocal(den) -> broadcast via matmul -> numhat = num * recip
  5. proj: out = sum_pairs numhat^T.T @ Wp_pad + bias, DMA out
"""

import os

import numpy as np

import concourse.bacc as bacc
import concourse.mybir as mybir
import concourse.tile as tile
from concourse.bass_utils import run_bass_kernel_spmd

F = mybir.dt.float32
R = mybir.dt.float32r

NH = 8
HD = 48
WIN = 7
HALF = 3
T, HH, WW = 4, 32, 32
C = 384
NPOS = T * HH * WW
SCALE = HD ** -0.5
NEG = -300.0

# per j-tile (4 halo key rows each): (i_lo, span, mask_col_offset)
SPANS = [
    (0, 128, 192),
    (0, 256, 64),
    (64, 320, 0),
    (192, 320, 0),
    (320, 192, 0),
    (448, 64, 0),
]

_CACHE = {}
LAST_RESULT = None


def _build_nc():
    if "nc" in _CACHE:
        return _CACHE["nc"]
    nc = bacc.Bacc("TRN2", target_bir_lowering=False)

    d_xT = nc.dram_tensor("xT", [128, 3, 768], R, kind="ExternalInput")
    d_wqk = nc.dram_tensor("wqk", [128, 3, 8, 128], R, kind="ExternalInput")
    d_wv = nc.dram_tensor("wv", [128, 3, 384], R, kind="ExternalInput")
    d_wp = nc.dram_tensor("wp", [128, 4, 384], R, kind="ExternalInput")
    d_bp = nc.dram_tensor("bp", [1, 384], R, kind="ExternalInput")
    d_mask = nc.dram_tensor("mask", [128, 320], R, kind="ExternalInput")
    d_noob = nc.dram_tensor("noob", [1, 512], R, kind="ExternalInput")
    d_sel2 = nc.dram_tensor("sel2", [64, 64], R, kind="ExternalInput")
    d_bsel = nc.dram_tensor("bsel", [32, 128], F, kind="ExternalInput")
    d_id = nc.dram_tensor("ident", [128, 128], R, kind="ExternalInput")
    d_ones1 = nc.dram_tensor("ones1", [1, 128], R, kind="ExternalInput")
    d_zero1 = nc.dram_tensor("zero1", [1, 128], R, kind="ExternalInput")
    d_onesD = nc.dram_tensor("onesD", [1, 32], R, kind="ExternalInput")
    d_vt = nc.dram_tensor("vt", [128, 8, 16], R, kind="ExternalInput")
    d_out = nc.dram_tensor("out", [512, 384], F, kind="ExternalOutput")

    EXP = mybir.ActivationFunctionType.Exp

    with tile.TileContext(nc) as tc:
        with tc.tile_pool(name="singles", bufs=1) as S:
            xT = S.tile([128, 3, 768], R)
            wqk = S.tile([128, 3, 8, 128], R)
            wv = S.tile([128, 3, 384], R)
            wp = S.tile([128, 4, 384], R)
            bp = S.tile([1, 384], R)
            mask = S.tile([128, 320], R)
            noob = S.tile([1, 512], R)
            sel2 = S.tile([64, 64], R)
            bsel = S.tile([32, 128], F)
            ident = S.tile([128, 128], R)
            ones1 = S.tile([1, 128], R)
            zero1 = S.tile([1, 128], R)
            onesD = S.tile([1, 32], R)
            vt = S.tile([128, 8, 16], R)
            qT2 = S.tile([128, 4, 768], R)
            kT2 = S.tile([128, 4, 768], R)
            vaug = S.tile([128, 6, 8, 64], R)
            nhat = S.tile([128, 4, 512], R)

            for sb, dr in [
                (xT, d_xT), (wqk, d_wqk), (wv, d_wv), (wp, d_wp),
                (bp, d_bp), (mask, d_mask), (noob, d_noob), (sel2, d_sel2),
                (bsel, d_bsel), (ident, d_id), (ones1, d_ones1),
                (zero1, d_zero1), (onesD, d_onesD), (vt, d_vt),
            ]:
                nc.sync.dma_start(out=sb[:], in_=dr[:])

            # ---- phase 1: q^T/k^T per head-pair, v natural -----------
            # wqk m-tile 2*pr   = [q_{2pr} | q_{2pr+1}] (cols 0:48, 64:112)
            # wqk m-tile 2*pr+1 = [k_{2pr} | k_{2pr+1}]
            with tc.tile_pool(name="psA", bufs=2, space="PSUM") as psA:
                for pr in range(4):
                    for s, dst in ((0, qT2), (1, kT2)):
                        A = psA.tile([128, 512], F, tag="A")
                        B = psA.tile([128, 256], F, tag="B")
                        for k in range(3):
                            st, sp_ = (k == 0), (k == 2)
                            nc.tensor.matmul(A[:], wqk[:, k, 2 * pr + s, :],
                                             xT[:, k, 0:512], start=st, stop=sp_)
                            nc.tensor.matmul(B[:], wqk[:, k, 2 * pr + s, :],
                                             xT[:, k, 512:768], start=st, stop=sp_)
                        nc.scalar.copy(dst[:, pr, 0:512], A[:])
                        nc.vector.tensor_copy(dst[:, pr, 512:768], B[:])
                for pt in range(6):
                    V = psA.tile([128, 384], F, tag="V")
                    for k in range(3):
                        nc.tensor.matmul(V[:], xT[:, k, 128 * pt:128 * pt + 128],
                                         wv[:, k, :], start=(k == 0), stop=(k == 2))
                    nc.vector.tensor_copy(
                        vaug[:, pt, :, 0:48],
                        V[:].rearrange("p (h d) -> p h d", h=8))
                    nc.scalar.copy(vaug[:, pt, :, 48:64], vt[:])

            # ---- phases 2-4 per head-pair ----------------------------
            with tc.tile_pool(name="psS", bufs=3, space="PSUM") as psS, \
                 tc.tile_pool(name="psO", bufs=2, space="PSUM") as psO, \
                 tc.tile_pool(name="psM", bufs=1, space="PSUM") as psM, \
                 tc.tile_pool(name="sb2", bufs=3) as sb2, \
                 tc.tile_pool(name="sbn", bufs=2) as sbn:
                for pr in range(4):
                    eTs = []
                    for e in range(2):
                        h = 2 * pr + e
                        eT = sb2.tile([128, 6, 320], R, tag="eT")
                        eTs.append(eT)
                        for jt in range(6):
                            ilo, spn, mo = SPANS[jt]
                            ps = psS.tile([128, 512], F, tag="s")
                            nc.tensor.matmul(ps[:, 0:spn], ident[:],
                                             mask[:, mo:mo + spn],
                                             start=True, stop=False)
                            nc.tensor.matmul(
                                ps[:, 0:spn],
                                kT2[64 * e:64 * e + 64, pr,
                                    128 * jt:128 * (jt + 1)],
                                qT2[64 * e:64 * e + 64, pr,
                                    96 + ilo:96 + ilo + spn],
                                start=False, stop=True)
                            nc.scalar.activation(eT[:, jt, 0:spn], ps[:, 0:spn],
                                                 EXP, scale=SCALE)
                    oTs = []
                    for e in range(2):
                        h = 2 * pr + e
                        O = psO.tile([64, 512], F, tag="O")
                        nc.tensor.matmul(O[:], zero1[:, 0:64], noob[:],
                                         start=True, stop=False,
                                         skip_group_check=True)
                        for jt in range(6):
                            ilo, spn, mo = SPANS[jt]
                            nc.tensor.matmul(
                                O[:, ilo:ilo + spn],
                                vaug[:, jt, h, :],
                                eTs[e][:, jt, 0:spn],
                                start=False, stop=(jt == 5),
                                skip_group_check=True)
                        oT = sbn.tile([64, 512], R, tag=f"oT{e}")
                        if e == 0:
                            nc.scalar.copy(oT[:], O[:])
                        else:
                            nc.vector.tensor_copy(oT[:], O[:])
                        oTs.append(oT)
                    D = psM.tile([32, 512], F, tag="D")
                    nc.tensor.matmul(D[:], sel2[:, 0:32], oTs[0][:],
                                     start=True, stop=False)
                    nc.tensor.matmul(D[:], sel2[:, 32:64], oTs[1][:],
                                     start=False, stop=False)
                    nc.tensor.matmul(D[:], onesD[:], noob[:],
                                     start=False, stop=True)
                    rec = sbn.tile([32, 512], F, tag="rec")
                    nc.vector.reciprocal(rec[:], D[:])
                    Bc0 = psM.tile([64, 512], F, tag="Bc0")
                    Bc1 = psM.tile([64, 512], F, tag="Bc1")
                    nc.tensor.matmul(Bc0[:], bsel[:, 0:64], rec[:],
                                     start=True, stop=True)
                    nc.tensor.matmul(Bc1[:], bsel[:, 64:128], rec[:],
                                     start=True, stop=True)
                    nc.vector.tensor_mul(nhat[0:64, pr, :], oTs[0][:], Bc0[:])
                    tmp1 = sbn.tile([64, 512], R, tag="tmp1")
                    nc.vector.tensor_mul(tmp1[:], oTs[1][:], Bc1[:])
                    nc.sync.dma_start(out=nhat[64:128, pr, :], in_=tmp1[:])

            # ---- phase 5: projection + bias --------------------------
            with tc.tile_pool(name="psP", bufs=2, space="PSUM") as psP, \
                 tc.tile_pool(name="sbo", bufs=2) as sbo:
                for it in range(4):
                    P = psP.tile([128, 384], F, tag="P")
                    for pr in range(4):
                        nc.tensor.matmul(P[:], nhat[:, pr, 128 * it:128 * (it + 1)],
                                         wp[:, pr, :], start=(pr == 0), stop=False)
                    nc.tensor.matmul(P[:], ones1[:], bp[:],
                                     start=False, stop=True)
                    ot = sbo.tile([128, 384], F, tag="ot")
                    nc.scalar.copy(ot[:], P[:])
                    nc.sync.dma_start(out=d_out[128 * it:128 * (it + 1), :],
                                      in_=ot[:])

    nc.compile()
    _CACHE["nc"] = nc
    return nc


def _host_consts(w_qkv, w_proj, b_proj):
    wqk = np.zeros((128, 3, 8, 128), np.float32)
    for k in range(3):
        rows = slice(k * 128, (k + 1) * 128)
        for pr in range(4):
            for s in range(2):  # 0 = q block, 1 = k block
                off = 384 * s
                wqk[:, k, 2 * pr + s, 0:48] = \
                    w_qkv[rows, off + 48 * (2 * pr):off + 48 * (2 * pr) + 48]
                wqk[:, k, 2 * pr + s, 64:112] = \
                    w_qkv[rows, off + 48 * (2 * pr + 1):off + 48 * (2 * pr + 1) + 48]
    wv = np.ascontiguousarray(
        w_qkv[:, 768:1152].reshape(3, 128, 384))
    wvp = np.ascontiguousarray(np.transpose(
        w_qkv[:, 768:1152].reshape(3, 128, 384), (1, 0, 2)))
    wp = np.zeros((128, 4, 384), np.float32)
    for pr in range(4):
        wp[0:48, pr, :] = w_proj[96 * pr:96 * pr + 48, :]
        wp[64:112, pr, :] = w_proj[96 * pr + 48:96 * pr + 96, :]
    bp = b_proj.reshape(1, 384).astype(np.float32)

    mask = np.full((128, 320), NEG, np.float32)
    for r in range(4):
        for q in range(10):
            if r <= q <= r + 6:
                blk = np.full((32, 32), NEG, np.float32)
                xj, xi = np.meshgrid(np.arange(32), np.arange(32), indexing="ij")
                blk[np.abs(xj - xi) <= 3] = 0.0
                mask[32 * r:32 * r + 32, 32 * q:32 * q + 32] = blk
    noob = np.zeros((1, 512), np.float32)
    for qy in range(16):
        for qx in range(32):
            noob[0, 32 * qy + qx] = 7.0 * (max(0, 3 - qx) + max(0, qx - 28))
    sel2 = np.zeros((64, 64), np.float32)
    sel2[48, 0] = 1.0            # D row 0 = den of even head
    sel2[48, 2:32] = 1.0         # filler rows stay finite
    sel2[48, 33] = 1.0           # D row 1 = den of odd head (B block col 33-32=1)
    bsel = np.zeros((32, 128), np.float32)
    bsel[0, 0:64] = 1.0          # Bc0 = rec row 0 broadcast
    bsel[1, 64:128] = 1.0        # Bc1 = rec row 1 broadcast
    ident = np.eye(128, dtype=np.float32)
    ones1 = np.ones((1, 128), np.float32)
    zero1 = np.zeros((1, 128), np.float32)
    onesD = np.zeros((1, 32), np.float32)
    onesD[0, 0:2] = 1.0
    vt = np.zeros((128, 8, 16), np.float32)
    vt[:, :, 0] = 1.0
    return dict(wqk=wqk, wv=wvp, wp=wp, bp=bp, mask=mask, noob=noob,
                sel2=sel2, bsel=bsel, ident=ident, ones1=ones1,
                zero1=zero1, onesD=onesD, vt=vt)


def kernel(x, w_qkv, w_proj, b_proj, H=32, W=32):
    global LAST_RESULT
    x = np.asarray(x, np.float32)
    w_qkv = np.asarray(w_qkv, np.float32)
    w_proj = np.asarray(w_proj, np.float32)
    b_proj = np.asarray(b_proj, np.float32)
    assert x.shape == (1, NPOS, C) and int(H) == 32 and int(W) == 32

    nc = _build_nc()
    consts = _host_consts(w_qkv, w_proj, b_proj)

    x4 = x[0].reshape(T, HH, WW, C)
    in_maps = []
    for c in range(8):
        t, ry0 = c // 2, 16 * (c % 2)
        xh = np.zeros((24, WW, C), np.float32)
        lo, hi = ry0 - 3, ry0 + 21
        slo, shi = max(lo, 0), min(hi, HH)
        xh[slo - lo:shi - lo] = x4[t, slo:shi]
        xT = np.ascontiguousarray(
            xh.reshape(768, C).T.reshape(3, 128, 768).transpose(1, 0, 2))
        in_maps.append({"xT": xT, **consts})

    trace = bool(int(os.environ.get("TRACE", "0")))
    res = run_bass_kernel_spmd(nc, in_maps, core_ids=list(range(8)),
                               trace=trace)
    LAST_RESULT = res
    out = np.concatenate([res.results[c]["out"] for c in range(8)], axis=0)
    return out.reshape(1, NPOS, C)



# revision 7
# speedup vs baseline: 1.5832x; 1.5832x over previous
"""Local (7x7 windowed) attention Trainium2 kernel, v2 (bf16).

Problem: B=1, N=4096 (T=4, H=W=32), C=384, 8 heads x hd=48, window 7x7
zero-padded (reference semantics: padded keys score exactly 0 -> weight
exp(0), value 0).

Sharding: data-parallel over positions. 8 cores; core c owns t-slice
c//2, query rows [16*(c%2), 16*(c%2)+16) (512 queries). Each core
recomputes k/v for a 3-row halo (24 rows = 768 halo positions,
zero-padded outside the image, matching the reference's zero padding).

v2 design (vs v1): all matmuls bf16 (1 cyc/row at any N, keeps the PE
HAM clock warm; fp32 ran at 1.5-4 cyc/row and went cold), the window
mask is a multiplicative 0/1 bf16 mask applied to exp(S) on the DVE
(removes 96 mask-add matmuls from the PE), reciprocal on ACT instead of
DVE (13us -> 2us), noob/zero-init matmuls folded away via PSUM
has_written semantics, phases reordered so the PE instruction stream
never waits on ACT/DVE.

Pipeline per core:
  P1: qT = wq^T x^T (owned 512 cols), kT = wk^T x^T (768 halo cols),
      v natural (768, 384) -> vaug [keys, 8, 64] (col 48 = ones so the
      V-matmul also produces the softmax denominator in row 48).
  P2: per (head, key-tile jt): S = kT.T @ qT in PSUM (banded spans),
      exp on ACT -> eT bf16, eT *= mask01 on DVE (kills out-of-window).
  P3: O[64,512] += vaug.T @ eT over jt (row 48 = denominator).
  P4: per pr: oT sbuf <- O pair; D = den + noob (2 matmuls);
      rec = 1/D on ACT; Bc = broadcast rec (1 matmul); nhat = oT*Bc.
  P5: out = sum_pr nhat^T @ Wp + bias, DMA out.
"""

import os

import numpy as np
import ml_dtypes

import concourse.bacc as bacc
import concourse.mybir as mybir
import concourse.tile as tile
from concourse.bass_utils import run_bass_kernel_spmd

F = mybir.dt.float32
BF = mybir.dt.bfloat16
BF_NP = ml_dtypes.bfloat16

NH = 8
HD = 48
WIN = 7
HALF = 3
T, HH, WW = 4, 32, 32
C = 384
NPOS = T * HH * WW
SCALE = HD ** -0.5

# per key-tile jt (4 halo key rows each): (ilo, span) in owned-query
# coords, plus the packed column offsets in the S psum banks / eT sbuf.
# Bank packing (512 f32 per psum bank): b0 = jt0+jt1+jt5 (448), b1 =
# jt2+jt4 (512), b2 = jt3 (320).
SPANS = [
    (0, 0, 128),    # jt0
    (1, 0, 256),    # jt1
    (2, 64, 320),   # jt2
    (3, 192, 320),  # jt3
    (4, 320, 192),  # jt4
    (5, 448, 64),   # jt5
]
# issue order groups by psum bank chunk so exp/mask can fire early
JT_ORDER = [0, 1, 5, 2, 4, 3]
S_OFF = {0: 0, 1: 128, 5: 384, 2: 512, 4: 832, 3: 1024}
E_OFF = {0: 0, 1: 128, 5: 384, 2: 448, 4: 768, 3: 960}
# exp/mask chunks: (s_lo, s_hi, e_lo, e_hi)
CHUNKS = [(0, 448, 0, 448), (512, 1024, 448, 960), (1024, 1344, 960, 1280)]

_CACHE = {}
LAST_RESULT = None


def _build_nc():
    if "nc" in _CACHE:
        return _CACHE["nc"]
    nc = bacc.Bacc("TRN2", target_bir_lowering=False)

    d_xT = nc.dram_tensor("xT", [128, 3, 768], BF, kind="ExternalInput")
    d_wqk = nc.dram_tensor("wqk", [128, 3, 8, 128], BF, kind="ExternalInput")
    d_wv = nc.dram_tensor("wv", [128, 3, 384], BF, kind="ExternalInput")
    d_wp = nc.dram_tensor("wp", [128, 4, 384], BF, kind="ExternalInput")
    d_bp = nc.dram_tensor("bp", [1, 384], BF, kind="ExternalInput")
    d_m01 = nc.dram_tensor("m01", [128, 1280], BF, kind="ExternalInput")
    d_noob = nc.dram_tensor("noob", [1, 512], BF, kind="ExternalInput")
    d_sel2 = nc.dram_tensor("sel2", [128, 2], BF, kind="ExternalInput")
    d_ones2 = nc.dram_tensor("ones2", [1, 2], BF, kind="ExternalInput")
    d_bsel = nc.dram_tensor("bsel", [2, 128], BF, kind="ExternalInput")
    d_ones1 = nc.dram_tensor("ones1", [1, 128], BF, kind="ExternalInput")
    d_vtall = nc.dram_tensor("vtall", [128, 6, 8, 16], BF, kind="ExternalInput")
    d_out = nc.dram_tensor("out", [512, 384], F, kind="ExternalOutput")

    EXP = mybir.ActivationFunctionType.Exp
    LN = mybir.ActivationFunctionType.Ln

    with tile.TileContext(nc) as tc:
        with tc.tile_pool(name="singles", bufs=1) as S:
            xT = S.tile([128, 3, 768], BF)
            wqk = S.tile([128, 3, 8, 128], BF)
            wv = S.tile([128, 3, 384], BF)
            wp = S.tile([128, 4, 384], BF)
            bp = S.tile([1, 384], BF)
            m01 = S.tile([128, 1280], BF)
            noob = S.tile([1, 512], BF)
            sel2 = S.tile([128, 2], BF)
            ones2 = S.tile([1, 2], BF)
            bsel = S.tile([2, 128], BF)
            ones1 = S.tile([1, 128], BF)
            qT2 = S.tile([128, 4, 512], BF)
            kT2 = S.tile([128, 4, 768], BF)
            vaug = S.tile([128, 6, 8, 64], BF)
            eTall = S.tile([128, 8, 1280], BF)
            nhat = S.tile([128, 4, 512], BF)

            for sb, dr in [
                (xT, d_xT), (wqk, d_wqk), (wv, d_wv), (wp, d_wp),
                (bp, d_bp), (m01, d_m01), (noob, d_noob), (sel2, d_sel2),
                (ones2, d_ones2), (bsel, d_bsel), (ones1, d_ones1),
            ]:
                nc.sync.dma_start(out=sb[:], in_=dr[:])
            nc.sync.dma_start(out=vaug[:, :, :, 48:64], in_=d_vtall[:])

            # ---- P1: qT (owned 512), kT (halo 768), v natural --------
            with tc.tile_pool(name="psA", bufs=2, space="PSUM") as psA:
                for pr in range(4):
                    Q = psA.tile([128, 512], F, tag="Q")
                    K1 = psA.tile([128, 512], F, tag="K1")
                    K2 = psA.tile([128, 256], F, tag="K2")
                    for k in range(3):
                        st, sp_ = (k == 0), (k == 2)
                        nc.tensor.matmul(Q[:], wqk[:, k, 2 * pr, :],
                                         xT[:, k, 96:608], start=st, stop=sp_)
                        nc.tensor.matmul(K1[:], wqk[:, k, 2 * pr + 1, :],
                                         xT[:, k, 0:512], start=st, stop=sp_)
                        nc.tensor.matmul(K2[:], wqk[:, k, 2 * pr + 1, :],
                                         xT[:, k, 512:768], start=st, stop=sp_)
                    nc.scalar.copy(qT2[:, pr, :], Q[:])
                    nc.scalar.copy(kT2[:, pr, 0:512], K1[:])
                    nc.vector.tensor_copy(kT2[:, pr, 512:768], K2[:])
                for pt in range(6):
                    V = psA.tile([128, 384], F, tag="V")
                    for k in range(3):
                        nc.tensor.matmul(V[:], xT[:, k, 128 * pt:128 * pt + 128],
                                         wv[:, k, :], start=(k == 0), stop=(k == 2))
                    nc.vector.tensor_copy(
                        vaug[:, pt, :, 0:48],
                        V[:].rearrange("p (h d) -> p h d", h=8))

            # ---- P2: scores + exp + mask, all 8 (pr, e) pairs --------
            with tc.tile_pool(name="psS", bufs=1, space="PSUM") as psS:
                for pr in range(4):
                    for e in range(2):
                        h = 2 * pr + e
                        Sb = [psS.tile([128, 512], F, tag=f"S{e}b0",
                                       name=f"S{e}b0"),
                              psS.tile([128, 512], F, tag=f"S{e}b1",
                                       name=f"S{e}b1"),
                              psS.tile([128, 320], F, tag=f"S{e}b2",
                                       name=f"S{e}b2")]
                        for jt in JT_ORDER:
                            ilo, spn = SPANS[jt][1], SPANS[jt][2]
                            so = S_OFF[jt]
                            b, bo = (0, so) if so < 512 else \
                                ((1, so - 512) if so < 1024 else (2, so - 1024))
                            nc.tensor.matmul(
                                Sb[b][:, bo:bo + spn],
                                kT2[64 * e:64 * e + 64, pr,
                                    128 * jt:128 * (jt + 1)],
                                qT2[64 * e:64 * e + 64, pr, ilo:ilo + spn],
                                start=True, stop=True)
                        for ci, (slo, shi, elo, ehi) in enumerate(CHUNKS):
                            b, bo = (0, slo) if slo < 512 else \
                                ((1, slo - 512) if slo < 1024 else (2, slo - 1024))
                            w = shi - slo
                            nc.scalar.activation(
                                eTall[:, h, elo:ehi], Sb[b][:, bo:bo + w],
                                EXP, scale=SCALE)
                            nc.vector.tensor_mul(
                                eTall[:, h, elo:ehi], eTall[:, h, elo:ehi],
                                m01[:, elo:ehi])

            # ---- P3/P4/P5 interleaved ---------------------------------
            with tc.tile_pool(name="psO", bufs=1, space="PSUM") as psO, \
                 tc.tile_pool(name="psD", bufs=1, space="PSUM") as psD, \
                 tc.tile_pool(name="psB", bufs=1, space="PSUM") as psB, \
                 tc.tile_pool(name="psP", bufs=1, space="PSUM") as psP, \
                 tc.tile_pool(name="sbn", bufs=2) as sbn, \
                 tc.tile_pool(name="sbo", bufs=2) as sbo:
                Ps = [psP.tile([128, 384], F, tag=f"P{it}", name=f"P{it}")
                      for it in range(4)]
                oTs = {}
                recs = {}

                def v_mms(pr):
                    for e in range(2):
                        h = 2 * pr + e
                        O = psO.tile([128, 512], F, tag=f"O{e}")
                        for i, jt in enumerate(JT_ORDER):
                            ilo, spn = SPANS[jt][1], SPANS[jt][2]
                            nc.tensor.matmul(
                                O[0:64, ilo:ilo + spn],
                                vaug[:, jt, h, :],
                                eTall[:, h, E_OFF[jt]:E_OFF[jt] + spn],
                                start=(i == 0), stop=(i == 5),
                                skip_group_check=True)
                        oT = oTs[pr]
                        if e == 0:
                            nc.scalar.copy(oT[0:64, :], O[0:64, :])
                        else:
                            nc.vector.tensor_copy(oT[64:128, :], O[0:64, :])

                def d_mms(pr):
                    D = psD.tile([2, 512], F, tag="D")
                    nc.tensor.matmul(D[:], sel2[:], oTs[pr][:],
                                     start=True, stop=False)
                    nc.tensor.matmul(D[:], ones2[:], noob[:],
                                     start=False, stop=True)
                    # 1/D as exp(-ln(D)): Ln and Exp share an ACT table
                    # (bass blocks ACT Reciprocal; DVE reciprocal is slow).
                    lnD = sbn.tile([2, 512], F, tag="lnD")
                    nc.scalar.activation(lnD[:], D[:], LN)
                    rec = sbn.tile([2, 512], BF, tag="rec")
                    nc.scalar.activation(rec[:], lnD[:], EXP, scale=-1.0)
                    recs[pr] = rec

                def bc_nhat(pr):
                    Bc = psB.tile([128, 512], F, tag="Bc")
                    nc.tensor.matmul(Bc[:], bsel[:], recs[pr][:],
                                     start=True, stop=True)
                    nc.vector.tensor_mul(nhat[:, pr, :], oTs[pr][:], Bc[:])

                def p5_mms(pr):
                    for it in range(4):
                        nc.tensor.matmul(
                            Ps[it][:], nhat[:, pr, 128 * it:128 * (it + 1)],
                            wp[:, pr, :], start=(pr == 0), stop=False,
                            skip_group_check=True)

                for pr in range(4):
                    oTs[pr] = sbo.tile([128, 512], BF, tag=f"oT{pr % 2}",
                                       name=f"oT{pr}")
                v_mms(0)
                v_mms(1)
                d_mms(0)
                bc_nhat(0)
                v_mms(2)
                p5_mms(0)
                d_mms(1)
                bc_nhat(1)
                v_mms(3)
                p5_mms(1)
                d_mms(2)
                bc_nhat(2)
                p5_mms(2)
                d_mms(3)
                bc_nhat(3)
                p5_mms(3)
                for it in range(4):
                    nc.tensor.matmul(Ps[it][:], ones1[:], bp[:],
                                     start=False, stop=True,
                                     skip_group_check=True)
                    ot = sbo.tile([128, 384], F, tag="ot")
                    nc.scalar.copy(ot[:], Ps[it][:])
                    nc.sync.dma_start(out=d_out[128 * it:128 * (it + 1), :],
                                      in_=ot[:])

    nc.compile()
    _CACHE["nc"] = nc
    return nc


def _host_consts(w_qkv, w_proj, b_proj):
    wqk = np.zeros((128, 3, 8, 128), np.float32)
    for k in range(3):
        rows = slice(k * 128, (k + 1) * 128)
        for pr in range(4):
            for s in range(2):  # 0 = q block, 1 = k block
                off = 384 * s
                wqk[:, k, 2 * pr + s, 0:48] = \
                    w_qkv[rows, off + 48 * (2 * pr):off + 48 * (2 * pr) + 48]
                wqk[:, k, 2 * pr + s, 64:112] = \
                    w_qkv[rows, off + 48 * (2 * pr + 1):off + 48 * (2 * pr + 1) + 48]
    wvp = np.ascontiguousarray(np.transpose(
        w_qkv[:, 768:1152].reshape(3, 128, 384), (1, 0, 2)))
    wp = np.zeros((128, 4, 384), np.float32)
    for pr in range(4):
        wp[0:48, pr, :] = w_proj[96 * pr:96 * pr + 48, :]
        wp[64:112, pr, :] = w_proj[96 * pr + 48:96 * pr + 96, :]
    bp = b_proj.reshape(1, 384)

    # 0/1 window mask in eT layout: for jt, entry (k, q) is in-window iff
    # |key_halo_row - query_halo_row| <= 3 and |kx - qx| <= 3.
    m01 = np.zeros((128, 1280), np.float32)
    kk = np.arange(128)
    for jt, ilo, spn in SPANS:
        q = np.arange(ilo, ilo + spn)
        krow = 4 * jt + kk[:, None] // 32
        qrow = q[None, :] // 32 + 3
        kx = kk[:, None] % 32
        qx = q[None, :] % 32
        good = (np.abs(krow - qrow) <= 3) & (np.abs(kx - qx) <= 3)
        m01[:, E_OFF[jt]:E_OFF[jt] + spn] = good.astype(np.float32)

    noob = np.zeros((1, 512), np.float32)
    for qy in range(16):
        for qx in range(32):
            noob[0, 32 * qy + qx] = 7.0 * (max(0, 3 - qx) + max(0, qx - 28))
    sel2 = np.zeros((128, 2), np.float32)
    sel2[48, 0] = 1.0
    sel2[112, 1] = 1.0
    ones2 = np.ones((1, 2), np.float32)
    bsel = np.zeros((2, 128), np.float32)
    bsel[0, 0:64] = 1.0
    bsel[1, 64:128] = 1.0
    ones1 = np.ones((1, 128), np.float32)
    vtall = np.zeros((128, 6, 8, 16), np.float32)
    vtall[:, :, :, 0] = 1.0
    c = dict(wqk=wqk, wv=wvp, wp=wp, bp=bp, m01=m01, noob=noob,
             sel2=sel2, ones2=ones2, bsel=bsel, ones1=ones1, vtall=vtall)
    return {k: np.ascontiguousarray(v.astype(BF_NP)) for k, v in c.items()}


def kernel(x, w_qkv, w_proj, b_proj, H=32, W=32):
    global LAST_RESULT
    x = np.asarray(x, np.float32)
    w_qkv = np.asarray(w_qkv, np.float32)
    w_proj = np.asarray(w_proj, np.float32)
    b_proj = np.asarray(b_proj, np.float32)
    assert x.shape == (1, NPOS, C) and int(H) == 32 and int(W) == 32

    nc = _build_nc()
    consts = _host_consts(w_qkv, w_proj, b_proj)

    x4 = x[0].reshape(T, HH, WW, C)
    in_maps = []
    for c in range(8):
        t, ry0 = c // 2, 16 * (c % 2)
        xh = np.zeros((24, WW, C), np.float32)
        lo, hi = ry0 - 3, ry0 + 21
        slo, shi = max(lo, 0), min(hi, HH)
        xh[slo - lo:shi - lo] = x4[t, slo:shi]
        xT = np.ascontiguousarray(
            xh.reshape(768, C).T.reshape(3, 128, 768).transpose(1, 0, 2)
        ).astype(BF_NP)
        in_maps.append({"xT": xT, **consts})

    trace = bool(int(os.environ.get("TRACE", "0")))
    res = run_bass_kernel_spmd(nc, in_maps, core_ids=list(range(8)),
                               trace=trace)
    LAST_RESULT = res
    out = np.concatenate([res.results[c]["out"] for c in range(8)], axis=0)
    return out.reshape(1, NPOS, C)


# revision 9
# speedup vs baseline: 1.7228x; 1.0882x over previous
"""Local (7x7 windowed) attention Trainium2 kernel, v3 (bf16).

Problem: B=1, N=4096 (T=4, H=W=32), C=384, 8 heads x hd=48, window 7x7
zero-padded (reference semantics: padded keys score exactly 0 -> weight
exp(0), value 0).

Sharding: data-parallel over positions. 8 cores; core c owns t-slice
c//2, query rows [16*(c%2), 16*(c%2)+16) (512 queries). Each core
recomputes k/v for a 3-row halo (24 rows = 768 halo positions,
zero-padded outside the image, matching the reference's zero padding).

v3 notes (each from trace evidence):
 - all matmuls bf16: 1 cyc/row at any N; fp32 ran 1.5-4 cyc/row and let
   the PE HAM clock drop to 1.2 GHz.
 - window mask = multiplicative 0/1 bf16 mask on exp(S), split between
   DVE and GpSimd (PE additive-mask matmuls were ~10k wasted rows).
 - one big exp per head instead of 3-6 small ones: ACT costs ~390 ns
   fixed per instruction on HW.
 - q and k share one 3-bank PSUM tile so evacuation is one copy per pr.
 - noob (x-out-of-bounds exp(0) count) folded into the O->SBUF copy as
   a tensor_add, denominators gathered across all 4 pr into one [8,512]
   and inverted with one reciprocal_approx_fast (DVE reciprocal costs
   6.5 ns/col; ACT Reciprocal is blocked by bass).
 - input DMAs spread across queues (sync: xT; gpsimd: consts) - 16 DMAs
   on one queue serialized ~12 us of startup in v2.
"""

import os

import numpy as np
import ml_dtypes

import concourse.bacc as bacc
import concourse.mybir as mybir
import concourse.tile as tile
from concourse.bass_utils import run_bass_kernel_spmd

F = mybir.dt.float32
R = mybir.dt.float32r
BF = mybir.dt.bfloat16
BF_NP = ml_dtypes.bfloat16

NH = 8
HD = 48
T, HH, WW = 4, 32, 32
C = 384
NPOS = T * HH * WW
SCALE = HD ** -0.5

# per key-tile jt (4 halo key rows each): (jt, ilo, span) in owned-query
# coords. jt5's span is extended 64->128 so the packed S layout has no
# uninitialized gap (the extra (k,q) pairs are out-of-window -> masked).
SPANS = [
    (0, 0, 128),
    (1, 0, 256),
    (2, 64, 320),
    (3, 192, 320),
    (4, 320, 192),
    (5, 384, 128),
]
# packed column offsets inside the [128, 1344] S/eT layout
# (bank0: jt0,jt1,jt5 = 512; bank1: jt2,jt4 = 512; bank2: jt3 = 320)
S_OFF = {0: 0, 1: 128, 5: 384, 2: 512, 4: 832, 3: 1024}
EW = 1344

_CACHE = {}
LAST_RESULT = None


def _build_nc():
    if "nc" in _CACHE:
        return _CACHE["nc"]
    nc = bacc.Bacc("TRN2", target_bir_lowering=False)

    d_xT = nc.dram_tensor("xT", [128, 3, 768], BF, kind="ExternalInput")
    d_wqk = nc.dram_tensor("wqk", [128, 3, 8, 128], BF, kind="ExternalInput")
    d_wv = nc.dram_tensor("wv", [128, 3, 384], BF, kind="ExternalInput")
    d_wp = nc.dram_tensor("wp", [128, 4, 384], BF, kind="ExternalInput")
    d_bp = nc.dram_tensor("bp", [1, 384], BF, kind="ExternalInput")
    d_m01 = nc.dram_tensor("m01", [128, EW], BF, kind="ExternalInput")
    d_noobp = nc.dram_tensor("noobp", [64, 512], BF, kind="ExternalInput")
    d_sel8 = nc.dram_tensor("sel8", [128, 4, 8], BF, kind="ExternalInput")
    d_bsel = nc.dram_tensor("bsel", [8, 4, 128], R, kind="ExternalInput")
    d_ones1 = nc.dram_tensor("ones1", [1, 128], BF, kind="ExternalInput")
    d_vtall = nc.dram_tensor("vtall", [128, 6, 8, 16], BF, kind="ExternalInput")
    d_out = nc.dram_tensor("out", [512, 384], F, kind="ExternalOutput")

    EXP = mybir.ActivationFunctionType.Exp

    with tile.TileContext(nc) as tc:
        with tc.tile_pool(name="singles", bufs=1) as S:
            xT = S.tile([128, 3, 768], BF)
            wqk = S.tile([128, 3, 8, 128], BF)
            wv = S.tile([128, 3, 384], BF)
            wp = S.tile([128, 4, 384], BF)
            bp = S.tile([1, 384], BF)
            m01 = S.tile([128, EW], BF)
            noobp = S.tile([64, 512], BF)
            sel8 = S.tile([128, 4, 8], BF)
            bsel = S.tile([8, 4, 128], R)
            ones1 = S.tile([1, 128], BF)
            qkT2 = S.tile([128, 4, 1280], BF)
            vaug = S.tile([128, 6, 8, 64], BF)
            eTall = S.tile([128, 8, EW], BF)
            nhat = S.tile([128, 4, 512], BF)

            # xT on the sync queue (per-k slices so P1 starts early);
            # everything else on the gpsimd queue (cheap DMA issue).
            for k in range(3):
                nc.sync.dma_start(out=xT[:, k, :], in_=d_xT[:, k, :])
            for sb, dr in [
                (wqk, d_wqk), (wv, d_wv), (m01, d_m01), (noobp, d_noobp),
                (wp, d_wp), (sel8, d_sel8), (bsel, d_bsel),
                (ones1, d_ones1), (bp, d_bp),
            ]:
                nc.gpsimd.dma_start(out=sb[:], in_=dr[:])
            nc.gpsimd.dma_start(out=vaug[:, :, :, 48:64], in_=d_vtall[:])

            # ---- P1: q (owned 512) + k (halo 768) in one PSUM tile ----
            with tc.tile_pool(name="psA", bufs=2, space="PSUM") as psA:
                for pr in range(4):
                    QK = psA.tile([128, 1536], F, tag="QK")
                    for k in range(3):
                        st, sp_ = (k == 0), (k == 2)
                        nc.tensor.matmul(QK[:, 0:512], wqk[:, k, 2 * pr, :],
                                         xT[:, k, 96:608], start=st, stop=sp_)
                        nc.tensor.matmul(QK[:, 512:1024],
                                         wqk[:, k, 2 * pr + 1, :],
                                         xT[:, k, 0:512], start=st, stop=sp_)
                        nc.tensor.matmul(QK[:, 1024:1280],
                                         wqk[:, k, 2 * pr + 1, :],
                                         xT[:, k, 512:768], start=st, stop=sp_)
                    nc.scalar.copy(qkT2[:, pr, :], QK[:, 0:1280])
                for pt in range(6):
                    V = psA.tile([128, 384], F, tag="V")
                    for k in range(3):
                        nc.tensor.matmul(V[:], xT[:, k, 128 * pt:128 * pt + 128],
                                         wv[:, k, :], start=(k == 0), stop=(k == 2))
                    nc.vector.tensor_copy(
                        vaug[:, pt, :, 0:48],
                        V[:].rearrange("p (h d) -> p h d", h=8))

            # ---- P2: scores + exp + mask, all 8 (pr, e) pairs --------
            with tc.tile_pool(name="psS", bufs=1, space="PSUM") as psS:
                for pr in range(4):
                    for e in range(2):
                        h = 2 * pr + e
                        Sb = psS.tile([128, 1536], F, tag=f"S{e}",
                                      name=f"S{e}")
                        for jt, ilo, spn in SPANS:
                            so = S_OFF[jt]
                            nc.tensor.matmul(
                                Sb[:, so:so + spn],
                                qkT2[64 * e:64 * e + 64, pr,
                                     512 + 128 * jt:512 + 128 * (jt + 1)],
                                qkT2[64 * e:64 * e + 64, pr, ilo:ilo + spn],
                                start=True, stop=True)
                        nc.scalar.activation(eTall[:, h, :], Sb[:, 0:EW],
                                             EXP, scale=SCALE)
                        if e == 0:
                            nc.gpsimd.tensor_mul(eTall[:, h, :],
                                                 eTall[:, h, :], m01[:])
                        else:
                            nc.vector.tensor_mul(eTall[:, h, :],
                                                 eTall[:, h, :], m01[:])

            # ---- P3/P4/P5 ---------------------------------------------
            with tc.tile_pool(name="psO", bufs=1, space="PSUM") as psO, \
                 tc.tile_pool(name="psD", bufs=1, space="PSUM") as psD, \
                 tc.tile_pool(name="psB", bufs=1, space="PSUM") as psB, \
                 tc.tile_pool(name="psP", bufs=1, space="PSUM") as psP, \
                 tc.tile_pool(name="sbn", bufs=1) as sbn, \
                 tc.tile_pool(name="sbo", bufs=2) as sbo:
                P = psP.tile([128, 4, 512], F)
                D = psD.tile([8, 512], F)
                oTs = {}
                for pr in range(4):
                    oTs[pr] = sbo.tile([128, 512], BF, tag=f"oT{pr % 2}",
                                       name=f"oT{pr}")

                def v_mms(pr):
                    for e in range(2):
                        h = 2 * pr + e
                        O = psO.tile([128, 512], F, tag=f"O{e}", name=f"O{e}")
                        for i, (jt, ilo, spn) in enumerate(SPANS):
                            nc.tensor.matmul(
                                O[0:64, ilo:ilo + spn],
                                vaug[:, jt, h, :],
                                eTall[:, h, S_OFF[jt]:S_OFF[jt] + spn],
                                start=(i == 0), stop=(i == 5),
                                skip_group_check=True)
                        # evacuate + add noob into the denominator row 48
                        nc.vector.tensor_add(oTs[pr][64 * e:64 * e + 64, :],
                                             O[0:64, :], noobp[:])

                for pr in range(4):
                    v_mms(pr)
                for pr in range(4):
                    nc.tensor.matmul(D[:], sel8[:, pr, :], oTs[pr][:],
                                     start=(pr == 0), stop=(pr == 3),
                                     skip_group_check=True)
                recf = sbn.tile([8, 512], F)
                nc.vector.reciprocal_approx_fast(recf[:], D[:])
                recr = sbn.tile([8, 512], R)
                nc.vector.tensor_copy(recr[:], recf[:])
                for pr in range(4):
                    Bc = psB.tile([128, 512], F, tag="Bc", name="Bc")
                    nc.tensor.matmul(Bc[:], bsel[:, pr, :], recr[:],
                                     start=True, stop=True)
                    nc.vector.tensor_mul(nhat[:, pr, :], oTs[pr][:], Bc[:])
                    for it in range(4):
                        nc.tensor.matmul(
                            P[:, it, 0:384],
                            nhat[:, pr, 128 * it:128 * (it + 1)],
                            wp[:, pr, :], start=(pr == 0), stop=False,
                            skip_group_check=True)
                for it in range(4):
                    nc.tensor.matmul(P[:, it, 0:384], ones1[:], bp[:],
                                     start=False, stop=True,
                                     skip_group_check=True)
                    ot = sbo.tile([128, 384], F, tag="ot", name=f"ot{it}")
                    if it % 2 == 0:
                        nc.scalar.copy(ot[:], P[:, it, 0:384])
                    else:
                        nc.vector.tensor_copy(ot[:], P[:, it, 0:384])
                    nc.gpsimd.dma_start(out=d_out[128 * it:128 * (it + 1), :],
                                        in_=ot[:])

    nc.compile()
    _CACHE["nc"] = nc
    return nc


def _host_consts(w_qkv, w_proj, b_proj):
    wqk = np.zeros((128, 3, 8, 128), np.float32)
    for k in range(3):
        rows = slice(k * 128, (k + 1) * 128)
        for pr in range(4):
            for s in range(2):  # 0 = q block, 1 = k block
                off = 384 * s
                wqk[:, k, 2 * pr + s, 0:48] = \
                    w_qkv[rows, off + 48 * (2 * pr):off + 48 * (2 * pr) + 48]
                wqk[:, k, 2 * pr + s, 64:112] = \
                    w_qkv[rows, off + 48 * (2 * pr + 1):off + 48 * (2 * pr + 1) + 48]
    wvp = np.ascontiguousarray(np.transpose(
        w_qkv[:, 768:1152].reshape(3, 128, 384), (1, 0, 2)))
    wp = np.zeros((128, 4, 384), np.float32)
    for pr in range(4):
        wp[0:48, pr, :] = w_proj[96 * pr:96 * pr + 48, :]
        wp[64:112, pr, :] = w_proj[96 * pr + 48:96 * pr + 96, :]
    bp = b_proj.reshape(1, 384)

    # 0/1 window mask in the packed S/eT layout: entry (k, q) of tile jt
    # is in-window iff |key_halo_row - query_halo_row| <= 3 and
    # |kx - qx| <= 3.
    m01 = np.zeros((128, EW), np.float32)
    kk = np.arange(128)
    for jt, ilo, spn in SPANS:
        q = np.arange(ilo, ilo + spn)
        krow = 4 * jt + kk[:, None] // 32
        qrow = q[None, :] // 32 + 3
        kx = kk[:, None] % 32
        qx = q[None, :] % 32
        good = (np.abs(krow - qrow) <= 3) & (np.abs(kx - qx) <= 3)
        m01[:, S_OFF[jt]:S_OFF[jt] + spn] = good.astype(np.float32)

    # noob folded into the O->oT copy: row 48 (the denominator row) gets
    # the count of x-out-of-bounds keys (reference zero-pads -> exp(0)).
    noobp = np.zeros((64, 512), np.float32)
    for qy in range(16):
        for qx in range(32):
            noobp[48, 32 * qy + qx] = 7.0 * (max(0, 3 - qx) + max(0, qx - 28))
    sel8 = np.zeros((128, 4, 8), np.float32)
    for pr in range(4):
        sel8[48, pr, 2 * pr] = 1.0
        sel8[112, pr, 2 * pr + 1] = 1.0
    bsel = np.zeros((8, 4, 128), np.float32)
    for pr in range(4):
        bsel[2 * pr, pr, 0:64] = 1.0
        bsel[2 * pr + 1, pr, 64:128] = 1.0
    ones1 = np.ones((1, 128), np.float32)
    vtall = np.zeros((128, 6, 8, 16), np.float32)
    vtall[:, :, :, 0] = 1.0
    c = dict(wqk=wqk, wv=wvp, wp=wp, bp=bp, m01=m01, noobp=noobp,
             sel8=sel8, ones1=ones1, vtall=vtall)
    out = {k: np.ascontiguousarray(v.astype(BF_NP)) for k, v in c.items()}
    out["bsel"] = np.ascontiguousarray(bsel)  # fp32r stays fp32 bits
    return out


def kernel(x, w_qkv, w_proj, b_proj, H=32, W=32):
    global LAST_RESULT
    x = np.asarray(x, np.float32)
    w_qkv = np.asarray(w_qkv, np.float32)
    w_proj = np.asarray(w_proj, np.float32)
    b_proj = np.asarray(b_proj, np.float32)
    assert x.shape == (1, NPOS, C) and int(H) == 32 and int(W) == 32

    nc = _build_nc()
    consts = _host_consts(w_qkv, w_proj, b_proj)

    x4 = x[0].reshape(T, HH, WW, C)
    in_maps = []
    for c in range(8):
        t, ry0 = c // 2, 16 * (c % 2)
        xh = np.zeros((24, WW, C), np.float32)
        lo, hi = ry0 - 3, ry0 + 21
        slo, shi = max(lo, 0), min(hi, HH)
        xh[slo - lo:shi - lo] = x4[t, slo:shi]
        xT = np.ascontiguousarray(
            xh.reshape(768, C).T.reshape(3, 128, 768).transpose(1, 0, 2)
        ).astype(BF_NP)
        in_maps.append({"xT": xT, **consts})

    trace = bool(int(os.environ.get("TRACE", "0")))
    res = run_bass_kernel_spmd(nc, in_maps, core_ids=list(range(8)),
                               trace=trace)
    LAST_RESULT = res
    out = np.concatenate([res.results[c]["out"] for c in range(8)], axis=0)
    return out.reshape(1, NPOS, C)


# revision 15
# speedup vs baseline: 1.8461x; 1.0715x over previous
"""Local (7x7 windowed) attention Trainium2 kernel, v3 (bf16).

Problem: B=1, N=4096 (T=4, H=W=32), C=384, 8 heads x hd=48, window 7x7
zero-padded (reference semantics: padded keys score exactly 0 -> weight
exp(0), value 0).

Sharding: data-parallel over positions. 8 cores; core c owns t-slice
c//2, query rows [16*(c%2), 16*(c%2)+16) (512 queries). Each core
recomputes k/v for a 3-row halo (24 rows = 768 halo positions,
zero-padded outside the image, matching the reference's zero padding).

v3 notes (each from trace evidence):
 - all matmuls bf16: 1 cyc/row at any N; fp32 ran 1.5-4 cyc/row and let
   the PE HAM clock drop to 1.2 GHz.
 - window mask = multiplicative 0/1 bf16 mask on exp(S), split between
   DVE and GpSimd (PE additive-mask matmuls were ~10k wasted rows).
 - one big exp per head instead of 3-6 small ones: ACT costs ~390 ns
   fixed per instruction on HW.
 - q and k share one 3-bank PSUM tile so evacuation is one copy per pr.
 - noob (x-out-of-bounds exp(0) count) folded into the O->SBUF copy as
   a tensor_add, denominators gathered across all 4 pr into one [8,512]
   and inverted with one reciprocal_approx_fast (DVE reciprocal costs
   6.5 ns/col; ACT Reciprocal is blocked by bass).
 - input DMAs spread across queues (sync: xT; gpsimd: consts) - 16 DMAs
   on one queue serialized ~12 us of startup in v2.
"""

import os

import numpy as np
import ml_dtypes

import concourse.bacc as bacc
import concourse.mybir as mybir
import concourse.tile as tile
from concourse.bass_utils import run_bass_kernel_spmd

F = mybir.dt.float32
R = mybir.dt.float32r
BF = mybir.dt.bfloat16
BF_NP = ml_dtypes.bfloat16

NH = 8
HD = 48
T, HH, WW = 4, 32, 32
C = 384
NPOS = T * HH * WW
SCALE = HD ** -0.5

# per key-tile jt (4 halo key rows each): (jt, ilo, span) in owned-query
# coords. jt5's span is extended 64->128 so the packed S layout has no
# uninitialized gap (the extra (k,q) pairs are out-of-window -> masked).
SPANS = [
    (0, 0, 128),
    (1, 0, 256),
    (2, 64, 320),
    (3, 192, 320),
    (4, 320, 192),
    (5, 384, 128),
]
# packed column offsets inside the [128, 1344] S/eT layout
# (bank0: jt0,jt1,jt5 = 512; bank1: jt2,jt4 = 512; bank2: jt3 = 320)
S_OFF = {0: 0, 1: 128, 5: 384, 2: 512, 4: 832, 3: 1024}
EW = 1344

_CACHE = {}
LAST_RESULT = None


def _build_nc():
    if "nc" in _CACHE:
        return _CACHE["nc"]
    nc = bacc.Bacc("TRN2", target_bir_lowering=False)

    d_xT = nc.dram_tensor("xT", [128, 3, 768], BF, kind="ExternalInput")
    d_wqk = nc.dram_tensor("wqk", [128, 3, 8, 128], BF, kind="ExternalInput")
    d_wv = nc.dram_tensor("wv", [128, 3, 384], BF, kind="ExternalInput")
    d_wp = nc.dram_tensor("wp", [128, 4, 384], BF, kind="ExternalInput")
    d_bp = nc.dram_tensor("bp", [1, 384], BF, kind="ExternalInput")
    d_mneg = nc.dram_tensor("mneg", [128, EW], BF, kind="ExternalInput")
    d_ident = nc.dram_tensor("ident", [128, 128], BF, kind="ExternalInput")
    d_noobp = nc.dram_tensor("noobp", [64, 512], BF, kind="ExternalInput")
    d_sel8 = nc.dram_tensor("sel8", [128, 4, 8], BF, kind="ExternalInput")
    d_bsel = nc.dram_tensor("bsel", [8, 4, 128], R, kind="ExternalInput")
    d_ones1 = nc.dram_tensor("ones1", [1, 128], BF, kind="ExternalInput")
    d_out = nc.dram_tensor("out", [512, 384], F, kind="ExternalOutput")

    EXP = mybir.ActivationFunctionType.Exp

    with tile.TileContext(nc) as tc:
        with tc.tile_pool(name="singles", bufs=1) as S:
            xT = S.tile([128, 3, 768], BF)
            wqk = S.tile([128, 3, 8, 128], BF)
            wv = S.tile([128, 3, 384], BF)
            wp = S.tile([128, 4, 384], BF)
            bp = S.tile([1, 384], BF)
            mneg = S.tile([128, EW], BF)
            ident = S.tile([128, 128], BF)
            noobp = S.tile([64, 512], BF)
            sel8 = S.tile([128, 4, 8], BF)
            bsel = S.tile([8, 4, 128], R)
            ones1 = S.tile([1, 128], BF)
            qkT2 = S.tile([128, 4, 1280], BF)
            vaug = S.tile([128, 6, 8, 64], BF)
            eTall = S.tile([128, 8, EW], BF)
            nhat = S.tile([128, 4, 512], BF)

            # Spread input DMAs over four queues so transfers overlap;
            # the first matmul needs only xT[k0] + wqk[k0] (both first on
            # sync). One 786KB wqk DMA measured ~7us at ~111GB/s, so wqk
            # is split per k-slice across queues.
            nc.sync.dma_start(out=xT[:, 0, :], in_=d_xT[:, 0, :])
            nc.sync.dma_start(out=wqk[:, 0, :, :], in_=d_wqk[:, 0, :, :])
            nc.sync.dma_start(out=xT[:, 1, :], in_=d_xT[:, 1, :])
            nc.sync.dma_start(out=xT[:, 2, :], in_=d_xT[:, 2, :])
            nc.scalar.dma_start(out=wqk[:, 1, :, :], in_=d_wqk[:, 1, :, :])
            nc.scalar.dma_start(out=wqk[:, 2, :, :], in_=d_wqk[:, 2, :, :])
            nc.scalar.dma_start(out=ident[:], in_=d_ident[:])
            nc.scalar.dma_start(out=wp[:], in_=d_wp[:])
            nc.scalar.dma_start(out=sel8[:], in_=d_sel8[:])
            nc.gpsimd.dma_start(out=wv[:], in_=d_wv[:])
            nc.gpsimd.dma_start(out=mneg[:], in_=d_mneg[:])
            nc.gpsimd.dma_start(out=noobp[:], in_=d_noobp[:])
            nc.gpsimd.dma_start(out=bsel[:], in_=d_bsel[:])
            nc.gpsimd.dma_start(out=ones1[:], in_=d_ones1[:])
            nc.gpsimd.dma_start(out=bp[:], in_=d_bp[:])
            # vaug's denominator-ones column (col 48) + zero pad 49:63
            nc.gpsimd.memset(vaug[:, :, :, 48:64], 0.0)
            nc.gpsimd.memset(vaug[:, :, :, 48:49], 1.0)

            # ---- P1: q (owned 512) + k (halo 768) in one PSUM tile ----
            with tc.tile_pool(name="psA", bufs=2, space="PSUM") as psA:
                for pr in range(4):
                    QK = psA.tile([128, 1536], F, tag="QK")
                    for k in range(3):
                        st, sp_ = (k == 0), (k == 2)
                        nc.tensor.matmul(QK[:, 0:512], wqk[:, k, 2 * pr, :],
                                         xT[:, k, 96:608], start=st, stop=sp_)
                        nc.tensor.matmul(QK[:, 512:1024],
                                         wqk[:, k, 2 * pr + 1, :],
                                         xT[:, k, 0:512], start=st, stop=sp_)
                        nc.tensor.matmul(QK[:, 1024:1280],
                                         wqk[:, k, 2 * pr + 1, :],
                                         xT[:, k, 512:768], start=st, stop=sp_)
                    nc.scalar.copy(qkT2[:, pr, :], QK[:, 0:1280])
                for pt in range(6):
                    V = psA.tile([128, 384], F, tag="V")
                    for k in range(3):
                        nc.tensor.matmul(V[:], xT[:, k, 128 * pt:128 * pt + 128],
                                         wv[:, k, :], start=(k == 0), stop=(k == 2))
                    nc.vector.tensor_copy(
                        vaug[:, pt, :, 0:48],
                        V[:].rearrange("p (h d) -> p h d", h=8))

            # ---- P2: mask-add + scores + exp, all 8 (pr, e) pairs -----
            # The -300 window mask is added on the PE: per pair, 6 mask
            # matmuls (ident stationary, loaded once) write the S tile,
            # then 6 score matmuls accumulate on top. start=True only on
            # the first matmul touching each PSUM bank (start clears the
            # whole bank's has_written bits; later mask matmuls then
            # overwrite their own region, score matmuls accumulate).
            BANK_FIRST = {0, 2, 3}  # jt whose region opens a new bank
            BANK_LAST = {5, 4, 3}   # jt whose score matmul closes a bank
            with tc.tile_pool(name="psS", bufs=1, space="PSUM") as psS:
                for pr in range(4):
                    for e in range(2):
                        h = 2 * pr + e
                        Sb = psS.tile([128, 1536], F, tag=f"S{e}",
                                      name=f"S{e}")
                        for jt, ilo, spn in SPANS:
                            so = S_OFF[jt]
                            nc.tensor.matmul(
                                Sb[:, so:so + spn], ident[:],
                                mneg[:, so:so + spn],
                                start=(jt in BANK_FIRST), stop=False,
                                skip_group_check=True)
                        for jt, ilo, spn in SPANS:
                            so = S_OFF[jt]
                            nc.tensor.matmul(
                                Sb[:, so:so + spn],
                                qkT2[64 * e:64 * e + 64, pr,
                                     512 + 128 * jt:512 + 128 * (jt + 1)],
                                qkT2[64 * e:64 * e + 64, pr, ilo:ilo + spn],
                                start=False, stop=(jt in BANK_LAST),
                                skip_group_check=True)
                        nc.scalar.activation(eTall[:, h, :], Sb[:, 0:EW],
                                             EXP, scale=SCALE)

            # ---- P3/P4/P5 ---------------------------------------------
            with tc.tile_pool(name="psO", bufs=1, space="PSUM") as psO, \
                 tc.tile_pool(name="psD", bufs=1, space="PSUM") as psD, \
                 tc.tile_pool(name="psB", bufs=1, space="PSUM") as psB, \
                 tc.tile_pool(name="psP", bufs=1, space="PSUM") as psP, \
                 tc.tile_pool(name="sbn", bufs=1) as sbn, \
                 tc.tile_pool(name="sbo", bufs=2) as sbo:
                P = psP.tile([128, 4, 512], F)
                D = psD.tile([8, 512], F)
                oTs = {}
                for pr in range(4):
                    oTs[pr] = sbo.tile([128, 512], BF, tag=f"oT{pr % 2}",
                                       name=f"oT{pr}")

                def v_mms(pr):
                    for e in range(2):
                        h = 2 * pr + e
                        O = psO.tile([128, 512], F, tag=f"O{e}", name=f"O{e}")
                        for i, (jt, ilo, spn) in enumerate(SPANS):
                            nc.tensor.matmul(
                                O[0:64, ilo:ilo + spn],
                                vaug[:, jt, h, :],
                                eTall[:, h, S_OFF[jt]:S_OFF[jt] + spn],
                                start=(i == 0), stop=(i == 5),
                                skip_group_check=True)
                        # evacuate + add noob into the denominator row 48
                        nc.vector.tensor_add(oTs[pr][64 * e:64 * e + 64, :],
                                             O[0:64, :], noobp[:])

                for pr in range(4):
                    v_mms(pr)
                for pr in range(4):
                    nc.tensor.matmul(D[:], sel8[:, pr, :], oTs[pr][:],
                                     start=(pr == 0), stop=(pr == 3),
                                     skip_group_check=True)
                recf = sbn.tile([8, 512], F)
                nc.vector.reciprocal_approx_fast(recf[:], D[:])
                recr = sbn.tile([8, 512], R)
                nc.vector.tensor_copy(recr[:], recf[:])
                for pr in range(4):
                    Bc = psB.tile([128, 512], F, tag="Bc", name="Bc")
                    nc.tensor.matmul(Bc[:], bsel[:, pr, :], recr[:],
                                     start=True, stop=True)
                    nc.vector.tensor_mul(nhat[:, pr, :], oTs[pr][:], Bc[:])
                    for it in range(4):
                        nc.tensor.matmul(
                            P[:, it, 0:384],
                            nhat[:, pr, 128 * it:128 * (it + 1)],
                            wp[:, pr, :], start=(pr == 0), stop=False,
                            skip_group_check=True)
                for it in range(4):
                    nc.tensor.matmul(P[:, it, 0:384], ones1[:], bp[:],
                                     start=False, stop=True,
                                     skip_group_check=True)
                    ot = sbo.tile([128, 384], F, tag="ot", name=f"ot{it}")
                    if it % 2 == 0:
                        nc.scalar.copy(ot[:], P[:, it, 0:384])
                    else:
                        nc.vector.tensor_copy(ot[:], P[:, it, 0:384])
                    nc.gpsimd.dma_start(out=d_out[128 * it:128 * (it + 1), :],
                                        in_=ot[:])

    nc.compile()
    _CACHE["nc"] = nc
    return nc


def _host_consts(w_qkv, w_proj, b_proj):
    wqk = np.zeros((128, 3, 8, 128), np.float32)
    for k in range(3):
        rows = slice(k * 128, (k + 1) * 128)
        for pr in range(4):
            for s in range(2):  # 0 = q block, 1 = k block
                off = 384 * s
                wqk[:, k, 2 * pr + s, 0:48] = \
                    w_qkv[rows, off + 48 * (2 * pr):off + 48 * (2 * pr) + 48]
                wqk[:, k, 2 * pr + s, 64:112] = \
                    w_qkv[rows, off + 48 * (2 * pr + 1):off + 48 * (2 * pr + 1) + 48]
    wvp = np.ascontiguousarray(np.transpose(
        w_qkv[:, 768:1152].reshape(3, 128, 384), (1, 0, 2)))
    wp = np.zeros((128, 4, 384), np.float32)
    for pr in range(4):
        wp[0:48, pr, :] = w_proj[96 * pr:96 * pr + 48, :]
        wp[64:112, pr, :] = w_proj[96 * pr + 48:96 * pr + 96, :]
    bp = b_proj.reshape(1, 384)

    # additive window mask in the packed S layout: entry (k, q) of tile
    # jt is in-window iff |key_halo_row - query_halo_row| <= 3 and
    # |kx - qx| <= 3; out-of-window scores get -300 (exp -> ~1e-19).
    mneg = np.zeros((128, EW), np.float32)
    kk = np.arange(128)
    for jt, ilo, spn in SPANS:
        q = np.arange(ilo, ilo + spn)
        krow = 4 * jt + kk[:, None] // 32
        qrow = q[None, :] // 32 + 3
        kx = kk[:, None] % 32
        qx = q[None, :] % 32
        good = (np.abs(krow - qrow) <= 3) & (np.abs(kx - qx) <= 3)
        mneg[:, S_OFF[jt]:S_OFF[jt] + spn] = np.where(good, 0.0, -300.0)

    # noob folded into the O->oT copy: row 48 (the denominator row) gets
    # the count of x-out-of-bounds keys (reference zero-pads -> exp(0)).
    noobp = np.zeros((64, 512), np.float32)
    for qy in range(16):
        for qx in range(32):
            noobp[48, 32 * qy + qx] = 7.0 * (max(0, 3 - qx) + max(0, qx - 28))
    sel8 = np.zeros((128, 4, 8), np.float32)
    for pr in range(4):
        sel8[48, pr, 2 * pr] = 1.0
        sel8[112, pr, 2 * pr + 1] = 1.0
    bsel = np.zeros((8, 4, 128), np.float32)
    for pr in range(4):
        bsel[2 * pr, pr, 0:64] = 1.0
        bsel[2 * pr + 1, pr, 64:128] = 1.0
    ones1 = np.ones((1, 128), np.float32)
    identm = np.eye(128, dtype=np.float32)
    c = dict(wqk=wqk, wv=wvp, wp=wp, bp=bp, mneg=mneg, noobp=noobp,
             sel8=sel8, ones1=ones1, ident=identm)
    out = {k: np.ascontiguousarray(v.astype(BF_NP)) for k, v in c.items()}
    out["bsel"] = np.ascontiguousarray(bsel)  # fp32r stays fp32 bits
    return out


def kernel(x, w_qkv, w_proj, b_proj, H=32, W=32):
    global LAST_RESULT
    x = np.asarray(x, np.float32)
    w_qkv = np.asarray(w_qkv, np.float32)
    w_proj = np.asarray(w_proj, np.float32)
    b_proj = np.asarray(b_proj, np.float32)
    assert x.shape == (1, NPOS, C) and int(H) == 32 and int(W) == 32

    nc = _build_nc()
    consts = _host_consts(w_qkv, w_proj, b_proj)

    x4 = x[0].reshape(T, HH, WW, C)
    in_maps = []
    for c in range(8):
        t, ry0 = c // 2, 16 * (c % 2)
        xh = np.zeros((24, WW, C), np.float32)
        lo, hi = ry0 - 3, ry0 + 21
        slo, shi = max(lo, 0), min(hi, HH)
        xh[slo - lo:shi - lo] = x4[t, slo:shi]
        xT = np.ascontiguousarray(
            xh.reshape(768, C).T.reshape(3, 128, 768).transpose(1, 0, 2)
        ).astype(BF_NP)
        in_maps.append({"xT": xT, **consts})

    trace = bool(int(os.environ.get("TRACE", "0")))
    res = run_bass_kernel_spmd(nc, in_maps, core_ids=list(range(8)),
                               trace=trace)
    LAST_RESULT = res
    out = np.concatenate([res.results[c]["out"] for c in range(8)], axis=0)
    return out.reshape(1, NPOS, C)
